# revision 37
# baseline (speedup 1.0000x reference)
"""Trainium2 Bass kernel for nn_Extinction (dense_mlp), 8-core data parallel.

Computation (per sample n):
  s_g(n)      = sigmoid(MLP_g(tpl[n, :2]))          for 6 gases g (2->6->4->4->1)
  out[n,c,k]  = cons[n,k] * exp(w_gas[k,c]) * mod   mod = 1 (k<2) else F[k-2,c]*s_{k-2}(n)

Strategy: shard N=524288 over 8 cores (65536 each). Per core, process blocks
of 2048 samples. All DRAM traffic is contiguous: tpl loads as [128, 48] and
cons as [128, 128] via the software DGE (gpsimd) so the SP HWDGE queue only
carries stores. The tpl tile is transposed on the PE into feature-major
[48, 128] (features r = 3b+i on partitions); layer 1 of the 6 tiny MLPs runs
as 4 chunk-matmuls whose [48, 72] weights fold the (sample-chunk -> stream)
gather into the weight matrix (an element-gather DMA for this layout measures
~7x slower than modeled on HW - avoid). Layers 2-4 run in the 4-stream
block-diagonal packing. All matmuls are bf16 (1 PE cycle/row vs 4 for f32);
layer-1 relus run on the DVE (tensor_scalar add+max), the rest on the scalar
engine, all writing bf16 SBUF tiles. PE transposes bring the sigmoid outputs
back to sample-major [128, 96]. The big elementwise product runs on the
VectorEngine entirely in bf16 (2x DVE mode, half the SBUF traffic); the
[128, 3840] bf16 out tile is stored by a casting SWDGE DMA that widens to
f32 on the way to DRAM (rel err ~0.6% vs the 2e-2 gate). Measured ~215 us
(200-240 across sessions) per NEFF execution vs the 472 us fp32 baseline;
the pure-store DMA floor measures ~167 us.

Sample <-> layout mapping within a block (base = blk*2048):
  n = base + 16*p + b     p = partition 0..127, b = 0..15   (DMA layout)
  b = 4*s + j             s = MLP stream 0..3, j = chunk 0..3
  MLP column q = 128*j + p  within stream s  ->  n = base + 16p + 4s + j
"""

import numpy as np
from ml_dtypes import bfloat16

N_TOTAL = 524288
N_CORES = 8
NS = N_TOTAL // N_CORES   # 65536 samples per core
NCH = 30
NK = 8
BLK = 2048                # samples per pipeline block
NBLK = NS // BLK          # 32
ROW = NCH * NK            # 240 outputs per sample
VERSION = 2               # bump on any kernel change: salts NEFF-cache shapes

# Per-gas channel filters (module constants of the reference nn.Module).
FILTERS = np.array([
    [1,1,1,1,1,1,1,1,1,1,1,1,1,1,1,1,1,1,1,1,1,1,0,0,0,0,0,0,1,1],  # h2o
    [1,1,0,0,0,0,0,0,0,0,0,0,0,0,0,0,1,1,1,1,1,1,0,0,1,1,1,1,1,1],  # o3
    [1,1,0,0,1,1,0,0,1,1,0,0,1,1,0,0,0,0,0,0,0,0,0,0,0,0,0,0,1,1],  # co2
    [1,1,0,0,0,0,0,0,0,0,0,0,0,0,1,1,1,1,1,1,1,1,0,0,0,0,1,1,1,1],  # u
    [1,1,0,0,1,0,0,0,0,0,0,0,0,0,0,0,0,0,0,0,0,0,0,0,0,0,0,0,1,1],  # n2o
    [1,1,1,1,0,0,1,1,0,0,1,1,0,0,0,0,0,0,0,0,0,0,0,0,0,0,0,0,1,1],  # ch4
], dtype=np.float32)


def prep_consts(w_gas, ke_W1, ke_b1, ke_W2, ke_b2, ke_W3, ke_b3, ke_Wo, ke_bo):
    """Pack the tiny weights for the 4-stream block-diagonal MLP (bf16)."""
    f32 = np.float32
    w_gas, ke_W1, ke_b1, ke_W2, ke_b2, ke_W3, ke_b3, ke_Wo, ke_bo = [
        np.asarray(a, f32)
        for a in (w_gas, ke_W1, ke_b1, ke_W2, ke_b2, ke_W3, ke_b3, ke_Wo, ke_bo)
    ]
    # Layer-1 weights with the chunk->stream gather folded in:
    # w1A[3b+i, 72j + 18s+6g+h] = ke_W1[g,h,i] for b = 4s+j (gases 0..2)
    w1A = np.zeros((48, 288), f32)
    w1B = np.zeros((48, 288), f32)
    w1aT = np.zeros((8, 72), f32)
    w1bT = np.zeros((8, 72), f32)
    b1a = np.zeros((72, 1), f32)
    b1b = np.zeros((72, 1), f32)
    w2aT = np.zeros((72, 96), f32)
    w2bT = np.zeros((72, 96), f32)
    b2 = np.zeros((96, 1), f32)
    w3T = np.zeros((96, 96), f32)
    b3 = np.zeros((96, 1), f32)
    woT = np.zeros((96, 24), f32)
    bo = np.zeros((24, 1), f32)
    for s in range(4):
        for j in range(4):
            b = 4 * s + j
            for g in range(3):
                for h in range(6):
                    col = 72 * j + 18 * s + 6 * g + h
                    for i in range(2):
                        w1A[3 * b + i, col] = ke_W1[g, h, i]
                        w1B[3 * b + i, col] = ke_W1[g + 3, h, i]
        for g in range(3):
            for h in range(6):
                r = 18 * s + 6 * g + h
                for i in range(2):
                    w1aT[4 * i + s, r] = ke_W1[g, h, i]
                    w1bT[4 * i + s, r] = ke_W1[g + 3, h, i]
                b1a[r, 0] = ke_b1[g, h]
                b1b[r, 0] = ke_b1[g + 3, h]
                for o in range(4):
                    w2aT[r, 24 * s + 4 * g + o] = ke_W2[g, o, h]
                    w2bT[r, 24 * s + 4 * (g + 3) + o] = ke_W2[g + 3, o, h]
        for g in range(6):
            for o in range(4):
                b2[24 * s + 4 * g + o, 0] = ke_b2[g, o]
                b3[24 * s + 4 * g + o, 0] = ke_b3[g, o]
                for h in range(4):
                    w3T[24 * s + 4 * g + h, 24 * s + 4 * g + o] = ke_W3[g, o, h]
                woT[24 * s + 4 * g + o, 6 * s + g] = ke_Wo[g, o]
            bo[6 * s + g, 0] = ke_bo[g]
    # EF[c*8+k] = exp(w_gas[k,c]) * (1 if k<2 else FILTERS[k-2,c])
    e = np.exp(w_gas)                      # [8, 30]
    ef_row = np.empty((NCH, NK), f32)
    for k in range(NK):
        m = 1.0 if k < 2 else FILTERS[k - 2]
        ef_row[:, k] = e[k] * m
    ef = np.tile(ef_row.reshape(1, ROW), (128, 1)).astype(f32)   # [128, 240]
    bf = bfloat16
    return {
        "w1A": w1A.astype(bf), "w1B": w1B.astype(bf),
        "w1aT": w1aT.astype(bf), "w1bT": w1bT.astype(bf),
        "b1a": b1a, "b1b": b1b,
        "w2aT": w2aT.astype(bf), "w2bT": w2bT.astype(bf), "b2": b2,
        "w3T": w3T.astype(bf), "b3": b3, "woT": woT.astype(bf), "bo": bo,
        "ef": ef,
        "efh": ef.astype(bf),
        "ident24": np.eye(24, dtype=f32),
        "ident128": np.eye(128, dtype=f32),
    }


CONST_SPECS = {
    "w1A": ((48, 288), "bf16"), "w1B": ((48, 288), "bf16"),
    "w1aT": ((8, 72), "bf16"), "w1bT": ((8, 72), "bf16"),
    "b1a": ((72, 1), "f32"), "b1b": ((72, 1), "f32"),
    "w2aT": ((72, 96), "bf16"), "w2bT": ((72, 96), "bf16"),
    "b2": ((96, 1), "f32"),
    "w3T": ((96, 96), "bf16"), "b3": ((96, 1), "f32"),
    "woT": ((96, 24), "bf16"), "bo": ((24, 1), "f32"),
    "ef": ((128, ROW), "f32"), "efh": ((128, ROW), "bf16"),
    "ident24": ((24, 24), "f32"), "ident128": ((128, 128), "f32"),
}


def build_program(nblk=NBLK, iters=1, tload=True, store=True, bigmul=True,
                  mlp=True, timing=False, salt=0,
                  io_bufs=6, mlp_bufs=6, big_bufs=6,
                  pmm_bufs=4, ptr_bufs=3, ptt_bufs=1, nsplit=4,
                  swdge_loads=True, bf16out=True, l1gather=False,
                  relu_dve=True, ttcopy_dve=True):
    """Build the per-core Bass program. Returns compiled nc.

    iters > 1 replicates the whole body inside the NEFF (same DRAM in/out)
    for steady-state throughput timing; results are unchanged.
    tload/store/bigmul/mlp=False ablate pieces (results become garbage) for
    bottleneck isolation on hardware.
    timing=True redirects stores to an internal DRAM scratch tensor and
    shrinks the ExternalOutput to [128, 4] so per-dispatch host<->device
    traffic is tiny (the 503MB output operand otherwise dominates wall time).
    """
    import concourse.bacc as bacc
    import concourse.mybir as mybir
    import concourse.tile as tile

    f32 = mybir.dt.float32
    bf16 = mybir.dt.bfloat16
    AF = mybir.ActivationFunctionType
    DT = {"f32": f32, "bf16": bf16}

    nc = bacc.Bacc("TRN2", target_bir_lowering=False, debug=False,
                   num_devices=N_CORES)
    ns = nblk * BLK
    tpl = nc.dram_tensor("tpl", [ns, 3], f32, kind="ExternalInput").ap()
    cons = nc.dram_tensor("cons", [ns, 8], f32, kind="ExternalInput").ap()
    cst = {
        k: nc.dram_tensor(k, list(sh), DT[d], kind="ExternalInput").ap()
        for k, (sh, d) in CONST_SPECS.items()
    }
    if timing:
        # salt the output shape: the NEFF cache keys on HLO shapes, not the
        # embedded BIR, so distinct variants must differ in shape
        salt = salt + 64 * VERSION
        out = nc.dram_tensor("out", [128 + salt, 4], f32,
                             kind="ExternalOutput").ap()
        out_f = None
    else:
        out = nc.dram_tensor("out", [ns, ROW], f32, kind="ExternalOutput").ap()
        out_f = out.flatten()

    tpl_f = tpl.flatten()
    cons_f = cons.flatten()

    with tile.TileContext(nc) as tc:
        with (
            tc.tile_pool(name="const", bufs=1) as cpool,
            tc.tile_pool(name="io", bufs=io_bufs) as iopool,
            tc.tile_pool(name="mlp", bufs=mlp_bufs) as mpool,
            tc.tile_pool(name="big", bufs=big_bufs) as bigpool,
            tc.tile_pool(name="pmm", bufs=pmm_bufs, space="PSUM") as pmm,
            tc.tile_pool(name="ptr", bufs=ptr_bufs, space="PSUM") as ptr,
            tc.tile_pool(name="ptt", bufs=ptt_bufs, space="PSUM") as ptt,
            tc.tile_pool(name="odram", bufs=1, space="DRAM") as odram,
        ):
            scratch = {}
            if timing:
                for blk in range(nblk):
                    scratch[blk] = odram.tile([128, 16 * ROW], f32,
                                              name=f"od{blk}", tag=f"od{blk}")
            # load constants once
            c_sb = {}
            for k, (sh, d) in CONST_SPECS.items():
                t = cpool.tile(list(sh), DT[d], tag=k)
                nc.sync.dma_start(t[:], cst[k][:])
                c_sb[k] = t
            if not tload:
                t_const = cpool.tile([128, 48], f32, tag="t_const")
                nc.gpsimd.memset(t_const[:], 0.25)
            if not mlp:
                sn_const = cpool.tile([128, 96], f32, tag="sn_const")
                nc.gpsimd.memset(sn_const[:], 0.5)
            if not bigmul:
                ot_const = cpool.tile([128, 16 * ROW], f32, tag="ot_const")
                nc.gpsimd.memset(ot_const[:], 0.125)

            for blk in range(nblk * iters):
                base = (blk % nblk) * BLK

                # ---- input DMA (contiguous) ----
                # with bf16out the casting stores own the SWDGE/Pool path;
                # loads then go back to the (now quiet) SP HWDGE queue
                ldeng = nc.gpsimd if (swdge_loads and not bf16out) else nc.sync
                if l1gather:
                    t_sb = None
                elif tload:
                    t_sb = iopool.tile([128, 48], f32, tag="tsb")
                    ldeng.dma_start(
                        t_sb[:],
                        tpl_f[3 * base: 3 * (base + BLK)].rearrange(
                            "(p f) -> p f", p=128))
                else:
                    t_sb = t_const

                cons_t = iopool.tile([128, 128], f32, tag="cons")
                ldeng.dma_start(
                    cons_t[:],
                    cons_f[8 * base: 8 * (base + BLK)].rearrange(
                        "(p f) -> p f", p=128))

                # ---- MLP (feature-major, 4 streams packed, bf16) ----
                if not mlp:
                    ps_T = sn_const
                else:
                    h1a = pmm.tile([72, 512], f32, tag="mm")
                    h1b = pmm.tile([72, 512], f32, tag="mm")
                    if l1gather:
                        # feature-major xt via casting SWDGE gather
                        # xt[4i+s, 128j+p] = tpl[base+16p+4s+j, i] (bf16)
                        xt = mpool.tile([8, 512], bf16, tag="xt")
                        t4 = tpl_f[3 * base: 3 * (base + BLK)].rearrange(
                            "(p s j i) -> s j p i", p=128, s=4, j=4, i=3)
                        xtv = xt[:].rearrange("q (j p) -> q j p", j=4)
                        nc.gpsimd.dma_start(xtv[0:4], t4[:, :, :, 0])
                        nc.gpsimd.dma_start(xtv[4:8], t4[:, :, :, 1])
                        nc.tensor.matmul(h1a[:], c_sb["w1aT"][:], xt[:],
                                         start=True, stop=True)
                        nc.tensor.matmul(h1b[:], c_sb["w1bT"][:], xt[:],
                                         start=True, stop=True)
                    else:
                        # tpl tile -> feature-major [48, 128] bf16
                        tt_ps = ptt.tile([48, 128], f32, tag="tt")
                        nc.tensor.transpose(tt_ps[:], t_sb[:],
                                            c_sb["ident128"][:])
                        tt_sb = mpool.tile([48, 128], bf16, tag="ttsb")
                        if ttcopy_dve:
                            nc.vector.tensor_copy(tt_sb[:], tt_ps[:])
                        else:
                            nc.scalar.copy(tt_sb[:], tt_ps[:])

                        # layer 1: per-chunk matmuls, gather folded into W
                        for j in range(4):
                            nc.tensor.matmul(
                                h1a[:, 128 * j: 128 * (j + 1)],
                                c_sb["w1A"][:, 72 * j: 72 * (j + 1)],
                                tt_sb[:], start=True, stop=True)
                        for j in range(4):
                            nc.tensor.matmul(
                                h1b[:, 128 * j: 128 * (j + 1)],
                                c_sb["w1B"][:, 72 * j: 72 * (j + 1)],
                                tt_sb[:], start=True, stop=True)
                    h1a_sb = mpool.tile([72, 512], bf16, tag="h1a")
                    h1b_sb = mpool.tile([72, 512], bf16, tag="h1b")
                    if relu_dve:
                        nc.vector.tensor_scalar(
                            h1a_sb[:], h1a[:], c_sb["b1a"][:], 0.0,
                            mybir.AluOpType.add, mybir.AluOpType.max)
                        nc.vector.tensor_scalar(
                            h1b_sb[:], h1b[:], c_sb["b1b"][:], 0.0,
                            mybir.AluOpType.add, mybir.AluOpType.max)
                    else:
                        nc.scalar.activation(h1a_sb[:], h1a[:], AF.Relu,
                                             bias=c_sb["b1a"][:])
                        nc.scalar.activation(h1b_sb[:], h1b[:], AF.Relu,
                                             bias=c_sb["b1b"][:])

                    h2 = pmm.tile([96, 512], f32, tag="mm")
                    nc.tensor.matmul(h2[:], c_sb["w2aT"][:], h1a_sb[:],
                                     start=True, stop=False)
                    nc.tensor.matmul(h2[:], c_sb["w2bT"][:], h1b_sb[:],
                                     start=False, stop=True)
                    h2_sb = mpool.tile([96, 512], bf16, tag="h2")
                    nc.scalar.activation(h2_sb[:], h2[:], AF.Relu,
                                         bias=c_sb["b2"][:])

                    h3 = pmm.tile([96, 512], f32, tag="mm")
                    nc.tensor.matmul(h3[:], c_sb["w3T"][:], h2_sb[:],
                                     start=True, stop=True)
                    h3_sb = mpool.tile([96, 512], bf16, tag="h3")
                    nc.scalar.activation(h3_sb[:], h3[:], AF.Relu,
                                         bias=c_sb["b3"][:])

                    s3 = pmm.tile([24, 512], f32, tag="mm")
                    nc.tensor.matmul(s3[:], c_sb["woT"][:], h3_sb[:],
                                     start=True, stop=True)
                    s3_sb = mpool.tile([24, 512], f32, tag="s3")
                    nc.scalar.activation(s3_sb[:], s3[:], AF.Sigmoid,
                                         bias=c_sb["bo"][:])

                    # ---- transpose s to sample-major: T[p, 24j + 6s + g] ----
                    ps_T = ptr.tile([128, 96], f32, tag="T")
                    for j in range(4):
                        nc.tensor.transpose(
                            ps_T[:, 24 * j: 24 * (j + 1)],
                            s3_sb[:, 128 * j: 128 * (j + 1)],
                            c_sb["ident24"][:])

                odt = bf16 if bf16out else f32
                if bigmul:
                    # ---- q8[p, 8b+k] = cons * (s-mod for k>=2, 1 for k<2) --
                    q8 = iopool.tile([128, 128], odt, tag="q8")
                    q8v = q8[:].rearrange("p (s j k) -> p s j k", s=4, j=4)
                    cons4 = cons_t[:].rearrange("p (s j k) -> p s j k",
                                                s=4, j=4)
                    Tv = ps_T[:].rearrange("p (j s g) -> p s j g", j=4, s=4)
                    nc.vector.tensor_mul(q8v[:, :, :, 2:8],
                                         cons4[:, :, :, 2:8], Tv)
                    q8c = q8[:].rearrange("p (b k) -> p b k", b=16)
                    consc = cons_t[:].rearrange("p (b k) -> p b k", b=16)
                    nc.vector.tensor_copy(q8c[:, :, 0:2], consc[:, :, 0:2])

                    # ---- big product -> out tile [128, 3840] ----
                    out_t = bigpool.tile([128, 16 * ROW], odt, tag="out")
                    o3 = out_t[:].rearrange("p (b c k) -> p b c k",
                                            b=16, c=NCH)
                    efk = "efh" if bf16out else "ef"
                    q8b = q8[:].rearrange("p (b k) -> p b k", b=16).unsqueeze(
                        2).broadcast_to([128, 16, NCH, NK])
                    ef3 = c_sb[efk][:].rearrange("p (c k) -> p c k",
                                                 c=NCH).unsqueeze(
                        1).broadcast_to([128, 16, NCH, NK])
                    bchunk = 16 // nsplit
                    for v in range(nsplit):
                        bs = slice(v * bchunk, (v + 1) * bchunk)
                        nc.vector.tensor_mul(o3[:, bs], q8b[:, bs],
                                             ef3[:, bs])
                else:
                    out_t = ot_const

                # ---- store (casting SWDGE store when bf16out) ----
                if store:
                    steng = nc.gpsimd if bf16out else nc.sync
                    fchunk = 16 * ROW // nsplit
                    for v in range(nsplit):
                        fs = slice(v * fchunk, (v + 1) * fchunk)
                        if timing:
                            steng.dma_start(
                                scratch[blk % nblk][:, fs], out_t[:, fs])
                        else:
                            dst = out_f[
                                ROW * base: ROW * (base + BLK)].rearrange(
                                "(p f) -> p f", p=128)
                            steng.dma_start(dst[:, fs], out_t[:, fs])

            if timing:
                # tiny real output so the program has one
                nc.sync.dma_start(out[0:128, :], c_sb["ef"][:, 0:4])

    nc.compile()
    return nc


_CACHE = {}


def _get_program():
    if "nc" not in _CACHE:
        _CACHE["nc"] = build_program()
    return _CACHE["nc"]


def kernel(**inputs):
    from concourse.bass_utils import run_bass_kernel_spmd

    tpl = np.asarray(inputs["tpl"], np.float32)
    cons = np.asarray(inputs["cons"], np.float32)
    consts = prep_consts(
        inputs["w_gas"], inputs["ke_W1"], inputs["ke_b1"], inputs["ke_W2"],
        inputs["ke_b2"], inputs["ke_W3"], inputs["ke_b3"], inputs["ke_Wo"],
        inputs["ke_bo"])

    nc = _get_program()
    in_maps = []
    for c in range(N_CORES):
        m = {"tpl": np.ascontiguousarray(tpl[c * NS:(c + 1) * NS]),
             "cons": np.ascontiguousarray(cons[c * NS:(c + 1) * NS])}
        m.update(consts)
        in_maps.append(m)
    res = run_bass_kernel_spmd(nc, in_maps, core_ids=list(range(N_CORES)))
    out = np.concatenate([res.results[c]["out"] for c in range(N_CORES)], axis=0)
    return out.reshape(N_TOTAL, NCH, NK)
